# revision 2
# baseline (speedup 1.0000x reference)
"""AudioRNN (GRU H=64, input_size=1) Trainium2 kernel.

Full-input contract: kernel(**inputs) takes the complete arrays and returns
(out, states) exactly like the reference. Internally the batch dim (32) is
sharded 4-rows-per-core across 8 NeuronCores; each core runs its own
sequential GRU scan over T=32768 steps.

Per-core layout (hidden dim on partitions 0-63):
  states_buf [65, 4, Tc+1] SBUF : rows 0-63 h, row 64 = ones (bias trick);
      column block j holds h after step t0+j-1 (col 0 = carry-in).
  Gate preacts accumulate in PSUM via K=65 matmuls ([W^T; b] @ [h; 1]) plus
  K=1 matmuls for the x-projections.  sigmoid/tanh on ACT, elementwise on DVE.
"""

import sys

if "/opt/trn_rl_repo" not in sys.path:
    sys.path.insert(0, "/opt/trn_rl_repo")

import numpy as np

H = 64
B = 32
N_CORES = 8
B_LOC = B // N_CORES  # 4 rows per core

_cache = {}


def _build(T, Tc):
    """Build + compile the per-core Bass program for sequence length T, chunk Tc."""
    import concourse.bacc as bacc
    import concourse.bass as bass
    import concourse.mybir as mybir
    from concourse import tile

    dt = mybir.dt.float32
    Alu = mybir.AluOpType
    Act = mybir.ActivationFunctionType

    assert T % Tc == 0 and Tc % 128 == 0

    nc = bacc.Bacc()

    x_in = nc.declare_dram_parameter("x", [B_LOC, T], dt, isOutput=False)
    p_wr = nc.declare_dram_parameter("lhsT_r", [H + 1, H], dt, isOutput=False)
    p_wz = nc.declare_dram_parameter("lhsT_z", [H + 1, H], dt, isOutput=False)
    p_wn = nc.declare_dram_parameter("lhsT_n", [H + 1, H], dt, isOutput=False)
    p_wxr = nc.declare_dram_parameter("w_xr", [1, H], dt, isOutput=False)
    p_wxz = nc.declare_dram_parameter("w_xz", [1, H], dt, isOutput=False)
    p_wxn = nc.declare_dram_parameter("lhsT_xn", [2, H], dt, isOutput=False)
    p_wlin = nc.declare_dram_parameter("w_lin_col", [H, 1], dt, isOutput=False)
    p_wxo = nc.declare_dram_parameter("lhsT_xout", [2, 1], dt, isOutput=False)
    p_id = nc.declare_dram_parameter("ident", [H, H], dt, isOutput=False)

    out_d = nc.declare_dram_parameter("out", [B_LOC, T], dt, isOutput=True)
    st_d = nc.declare_dram_parameter("states", [B_LOC, T, H], dt, isOutput=True)

    with tile.TileContext(nc) as tc:
        with (
            tc.tile_pool(name="const", bufs=1) as cpool,
            tc.tile_pool(name="state", bufs=1) as spool,
            tc.tile_pool(name="work", bufs=2) as wpool,
            tc.tile_pool(name="stage", bufs=2) as stpool,
            tc.tile_pool(name="ps_g", bufs=2, space="PSUM") as pg,
            tc.tile_pool(name="ps_n", bufs=2, space="PSUM") as pn,
            tc.tile_pool(name="ps_misc", bufs=2, space="PSUM") as pm,
        ):
            # --- constants / weights ---
            Wr = cpool.tile([H + 1, H], dt, tag="Wr")
            Wz = cpool.tile([H + 1, H], dt, tag="Wz")
            Wn = cpool.tile([H + 1, H], dt, tag="Wn")
            wxr = cpool.tile([1, H], dt, tag="wxr")
            wxz = cpool.tile([1, H], dt, tag="wxz")
            wxn = cpool.tile([2, H], dt, tag="wxn")
            wlin = cpool.tile([H, 1], dt, tag="wlin")
            wxo = cpool.tile([2, 1], dt, tag="wxo")
            ident = cpool.tile([H, H], dt, tag="ident")
            nc.sync.dma_start(Wr[:, :], p_wr[:, :])
            nc.sync.dma_start(Wz[:, :], p_wz[:, :])
            nc.sync.dma_start(Wn[:, :], p_wn[:, :])
            nc.sync.dma_start(wxr[:, :], p_wxr[:, :])
            nc.sync.dma_start(wxz[:, :], p_wxz[:, :])
            nc.sync.dma_start(wxn[:, :], p_wxn[:, :])
            nc.sync.dma_start(wlin[:, :], p_wlin[:, :])
            nc.sync.dma_start(wxo[:, :], p_wxo[:, :])
            nc.sync.dma_start(ident[:, :], p_id[:, :])

            # --- persistent state ---
            sb = spool.tile([H + 1, B_LOC, Tc + 1], dt, tag="states_buf")
            xt = spool.tile([2, B_LOC, Tc], dt, tag="x_tile")
            xpn = spool.tile([H, B_LOC, Tc], dt, tag="xpn")
            nc.vector.memset(sb[H : H + 1, :, :], 1.0)  # ones row (bias trick)
            nc.vector.memset(sb[0:H, :, 0:1], 0.0)  # h0 = 0
            # ones row for x-side: memset whole tile (engine APs need 32-aligned
            # partition base); chunk DMA overwrites row 0 with x each iteration
            nc.vector.memset(xt[:, :, :], 1.0)

            with tc.For_i(0, T, Tc) as t0:
                # chunk x load
                for b in range(B_LOC):
                    nc.sync.dma_start(xt[0:1, b, :], x_in[b : b + 1, bass.ds(t0, Tc)])
                # xn = w_in * x + b_in, per batch row
                for b in range(B_LOC):
                    ps = pm.tile([H, Tc], dt, tag="misc")
                    nc.tensor.matmul(ps[:, :], wxn[:, :], xt[:, b, :], start=True, stop=True)
                    nc.scalar.copy(xpn[:, b, :], ps[:, :])

                # --- the sequential scan ---
                for j in range(Tc):
                    h1 = sb[:, :, j]          # [65, 4]  ([h; 1])
                    hprev = sb[0:H, :, j]     # [64, 4]
                    xj = xt[0:1, :, j]        # [1, 4]

                    psg = pg.tile([H, 2 * B_LOC], dt, tag="g")
                    nc.tensor.matmul(psg[:, 0:B_LOC], Wr[:, :], h1, start=True, stop=False)
                    nc.tensor.matmul(psg[:, 0:B_LOC], wxr[:, :], xj, start=False, stop=True)
                    nc.tensor.matmul(psg[:, B_LOC : 2 * B_LOC], Wz[:, :], h1, start=True, stop=False)
                    nc.tensor.matmul(psg[:, B_LOC : 2 * B_LOC], wxz[:, :], xj, start=False, stop=True)
                    psn = pn.tile([H, B_LOC], dt, tag="n")
                    nc.tensor.matmul(psn[:, :], Wn[:, :], h1, start=True, stop=True)

                    rz = wpool.tile([H, 2 * B_LOC], dt, tag="rz")
                    nc.scalar.activation(rz[:, :], psg[:, :], Act.Sigmoid)

                    m_ = wpool.tile([H, B_LOC], dt, tag="m")
                    nc.vector.tensor_tensor(m_[:, :], rz[:, 0:B_LOC], psn[:, :], Alu.mult)
                    a_ = wpool.tile([H, B_LOC], dt, tag="a")
                    nc.vector.tensor_tensor(a_[:, :], m_[:, :], xpn[:, :, j], Alu.add)
                    n_ = wpool.tile([H, B_LOC], dt, tag="nn")
                    nc.scalar.activation(n_[:, :], a_[:, :], Act.Tanh)

                    zp = wpool.tile([H, B_LOC], dt, tag="zp")
                    nc.vector.tensor_scalar(zp[:, :], rz[:, B_LOC : 2 * B_LOC], -1.0, 1.0, Alu.mult, Alu.add)
                    u_ = wpool.tile([H, B_LOC], dt, tag="u")
                    nc.vector.tensor_tensor(u_[:, :], rz[:, B_LOC : 2 * B_LOC], hprev, Alu.mult)
                    t3 = wpool.tile([H, B_LOC], dt, tag="t3")
                    nc.vector.tensor_tensor(t3[:, :], n_[:, :], zp[:, :], Alu.mult)
                    nc.vector.tensor_tensor(sb[0:H, :, j + 1], t3[:, :], u_[:, :], Alu.add)

                # --- linear head: out = w_lin . h + b_lin + x ---
                for b in range(B_LOC):
                    ps = pm.tile([1, Tc], dt, tag="misc")
                    nc.tensor.matmul(ps[:, :], wlin[:, :], sb[0:H, b, 1 : Tc + 1], start=True, stop=False)
                    nc.tensor.matmul(ps[:, :], wxo[:, :], xt[:, b, :], start=False, stop=True)
                    osb = stpool.tile([1, Tc], dt, tag="osb")
                    nc.scalar.copy(osb[:, :], ps[:, :])
                    nc.sync.dma_start(out_d[b : b + 1, bass.ds(t0, Tc)], osb[:, :])

                # --- states out: transpose [64, 128] -> [128, 64] and store ---
                for b in range(B_LOC):
                    for k in range(Tc // 128):
                        ps = pm.tile([128, H], dt, tag="misc")
                        nc.tensor.transpose(ps[:, :], sb[0:H, b, 1 + 128 * k : 1 + 128 * (k + 1)], ident[:, :])
                        ssb = stpool.tile([128, H], dt, tag="ssb")
                        nc.scalar.copy(ssb[:, :], ps[:, :])
                        nc.sync.dma_start(st_d[b, bass.ds(t0 + 128 * k, 128), :], ssb[:, :])

                # --- carry h across chunks ---
                nc.vector.tensor_copy(sb[0:H, :, 0], sb[0:H, :, Tc])

    nc.compile()
    return nc


def _get_nc(T, Tc):
    key = (T, Tc)
    if key not in _cache:
        _cache[key] = _build(T, Tc)
    return _cache[key]


def _prep_weights(w_ih, w_hh, b_ih, b_hh, w_lin, b_lin):
    w_ih = np.asarray(w_ih, np.float32)
    w_hh = np.asarray(w_hh, np.float32)
    b_ih = np.asarray(b_ih, np.float32)
    b_hh = np.asarray(b_hh, np.float32)
    w_lin = np.asarray(w_lin, np.float32)
    b_lin = np.asarray(b_lin, np.float32)

    def lhsT(rows, bias):
        # [H+1, H]: rows 0..H-1 = W^T (W = w_hh[rows]), row H = bias
        m = np.empty((H + 1, H), np.float32)
        m[0:H, :] = w_hh[rows, :].T
        m[H, :] = bias
        return m

    r, z, n = slice(0, H), slice(H, 2 * H), slice(2 * H, 3 * H)
    d = {
        "lhsT_r": lhsT(r, b_ih[r] + b_hh[r]),
        "lhsT_z": lhsT(z, b_ih[z] + b_hh[z]),
        "lhsT_n": lhsT(n, b_hh[n]),
        "w_xr": w_ih[r, 0][None, :].copy(),
        "w_xz": w_ih[z, 0][None, :].copy(),
        "lhsT_xn": np.stack([w_ih[n, 0], b_ih[n]]).astype(np.float32),
        "w_lin_col": w_lin[0][:, None].copy(),
        "lhsT_xout": np.array([[1.0], [b_lin[0]]], np.float32),
        "ident": np.eye(H, dtype=np.float32),
    }
    return {k: np.ascontiguousarray(v) for k, v in d.items()}


def _run(x, weights, T, Tc):
    from concourse.bass_utils import run_bass_kernel_spmd

    nc = _get_nc(T, Tc)
    in_maps = []
    for c in range(N_CORES):
        m = {"x": np.ascontiguousarray(x[c * B_LOC : (c + 1) * B_LOC])}
        m.update(weights)
        in_maps.append(m)
    res = run_bass_kernel_spmd(nc, in_maps, list(range(N_CORES)))
    out = np.concatenate([res.results[c]["out"] for c in range(N_CORES)], axis=0)
    states = np.concatenate([res.results[c]["states"] for c in range(N_CORES)], axis=0)
    return out, states


def kernel(x, w_ih, w_hh, b_ih, b_hh, w_lin, b_lin, _Tc=512):
    x = np.asarray(x, np.float32)
    T = x.shape[1]
    weights = _prep_weights(w_ih, w_hh, b_ih, b_hh, w_lin, b_lin)
    return _run(x, weights, T, _Tc)


# revision 19
# speedup vs baseline: 155.8441x; 155.8441x over previous
"""AudioRNN (GRU H=64, input_size=1) Trainium2 kernel.

Full-input contract: kernel(**inputs) takes the complete arrays and returns
(out, states) exactly like the reference. Internally the batch dim (32) is
sharded 4-rows-per-core across 8 NeuronCores; each core runs its own
sequential GRU scan over T=32768 steps.

Per-core layout (hidden dim on partitions 0-63):
  states_buf [65, 4, Tc+1] SBUF : rows 0-63 h, row 64 = ones (bias trick);
      column block j holds h after step t0+j-1 (col 0 = carry-in).
  Gate preacts accumulate in PSUM via K=65 matmuls ([W^T; b] @ [h; 1]) plus
  K=1 matmuls for the x-projections.  sigmoid/tanh on ACT, elementwise on DVE.
"""

import sys

if "/opt/trn_rl_repo" not in sys.path:
    sys.path.insert(0, "/opt/trn_rl_repo")

import numpy as np

H = 64
B = 32
N_CORES = 8
B_LOC = B // N_CORES  # 4 rows per core

_cache = {}
BEST_KW = {}  # tuned build options used by kernel()
_STRIP = 0  # perf experiments only: 1 = skip blend, 2 = skip m/a


def _build(T, Tc, epochs=1, internal_out=False, unroll=False, split=False, wbufs=2, pbufs=2):
    """Build + compile the per-core Bass program for sequence length T, chunk Tc.

    epochs>1 / internal_out=True are used only by the perf harness: the scan
    loop repeats `epochs` times and outputs go to internal DRAM so the host
    transfer cost disappears from wall-clock measurements."""
    import concourse.bacc as bacc
    import concourse.bass as bass
    import concourse.mybir as mybir
    from concourse import tile

    dt = mybir.dt.float32
    Alu = mybir.AluOpType
    Act = mybir.ActivationFunctionType

    assert T % Tc == 0 and Tc % 128 == 0

    nc = bacc.Bacc()

    x_in = nc.declare_dram_parameter("x", [B_LOC, T], dt, isOutput=False)
    p_wr = nc.declare_dram_parameter("lhsT_r", [H + 1, H], dt, isOutput=False)
    p_wz = nc.declare_dram_parameter("lhsT_z", [H + 1, H], dt, isOutput=False)
    p_wn = nc.declare_dram_parameter("lhsT_n", [H + 1, H], dt, isOutput=False)
    p_wxrz = nc.declare_dram_parameter("w_xrz", [2, H], dt, isOutput=False)
    p_wxn = nc.declare_dram_parameter("lhsT_xn", [2, H], dt, isOutput=False)
    p_wlin = nc.declare_dram_parameter("w_lin_col", [H, 1], dt, isOutput=False)
    p_wxo = nc.declare_dram_parameter("lhsT_xout", [2, 1], dt, isOutput=False)
    p_id = nc.declare_dram_parameter("ident", [H, H], dt, isOutput=False)

    if internal_out:
        ok_d = nc.declare_dram_parameter("ok", [1, 1], dt, isOutput=True)
        out_d = nc.dram_tensor("out_i", [B_LOC, T], dt)
        st_d = nc.dram_tensor("states_i", [B_LOC, T, H], dt)
    else:
        out_d = nc.declare_dram_parameter("out", [B_LOC, T], dt, isOutput=True)
        st_d = nc.declare_dram_parameter("states", [B_LOC, T, H], dt, isOutput=True)

    with tile.TileContext(nc) as tc:
        with (
            tc.tile_pool(name="const", bufs=1) as cpool,
            tc.tile_pool(name="state", bufs=1) as spool,
            tc.tile_pool(name="work", bufs=wbufs) as wpool,
            tc.tile_pool(name="stage", bufs=2) as stpool,
            tc.tile_pool(name="ps_g", bufs=pbufs, space="PSUM") as pg,
            tc.tile_pool(name="ps_n", bufs=pbufs, space="PSUM") as pn,
            tc.tile_pool(name="ps_misc", bufs=2, space="PSUM") as pm,
        ):
            # --- constants / weights ---
            Wr = cpool.tile([H + 1, H], dt, tag="Wr")
            Wz = cpool.tile([H + 1, H], dt, tag="Wz")
            Wn = cpool.tile([H + 1, H], dt, tag="Wn")
            wxrz = cpool.tile([2, H], dt, tag="wxrz")
            wxn = cpool.tile([2, H], dt, tag="wxn")
            wlin = cpool.tile([H, 1], dt, tag="wlin")
            wxo = cpool.tile([2, 1], dt, tag="wxo")
            ident = cpool.tile([H, H], dt, tag="ident")
            nc.sync.dma_start(Wr[:, :], p_wr[:, :])
            nc.sync.dma_start(Wz[:, :], p_wz[:, :])
            nc.sync.dma_start(Wn[:, :], p_wn[:, :])
            nc.sync.dma_start(wxrz[:, :], p_wxrz[:, :])
            nc.sync.dma_start(wxn[:, :], p_wxn[:, :])
            nc.sync.dma_start(wlin[:, :], p_wlin[:, :])
            nc.sync.dma_start(wxo[:, :], p_wxo[:, :])
            nc.sync.dma_start(ident[:, :], p_id[:, :])

            # --- persistent state ---
            sb = spool.tile([H + 1, B_LOC, Tc + 1], dt, tag="states_buf")
            xt = spool.tile([2, B_LOC, Tc], dt, tag="x_tile")
            xz = spool.tile([2, 2 * B_LOC, Tc], dt, tag="xz_tile")
            xpn = spool.tile([H, B_LOC, Tc], dt, tag="xpn")
            nc.vector.memset(sb[H : H + 1, :, :], 1.0)  # ones row (bias trick)
            nc.vector.memset(sb[0:H, :, 0:1], 0.0)  # h0 = 0
            # ones row for x-side: memset whole tile (engine APs need 32-aligned
            # partition base); chunk DMA overwrites row 0 with x each iteration
            nc.vector.memset(xt[:, :, :], 1.0)
            nc.vector.memset(xz[:, :, :], 0.0)
            if split:
                uts = [spool.tile([H + 1, B_LOC], dt, name=f"usp{i}", tag=f"u{i}") for i in range(2)]
                t3s = [spool.tile([H, B_LOC], dt, name=f"t3sp{i}", tag=f"t3{i}") for i in range(2)]
                for ut in uts:
                    nc.vector.memset(ut[:, :], 1.0)  # row 64 stays ones

            def chunk_body(t0):
                # chunk x load: xt rows (x, ones); xz rows (x|0, 0|x) for the
                # combined r/z x-projection matmul
                for b in range(B_LOC):
                    nc.sync.dma_start(xt[0:1, b, :], x_in[b : b + 1, bass.ds(t0, Tc)])
                    nc.sync.dma_start(xz[0:1, b, :], x_in[b : b + 1, bass.ds(t0, Tc)])
                    nc.sync.dma_start(xz[1:2, B_LOC + b, :], x_in[b : b + 1, bass.ds(t0, Tc)])
                # xn = w_in * x + b_in, per batch row
                for b in range(B_LOC):
                    ps = pm.tile([H, Tc], dt, tag="misc")
                    nc.tensor.matmul(ps[:, :], wxn[:, :], xt[:, b, :], start=True, stop=True)
                    nc.scalar.copy(xpn[:, b, :], ps[:, :])

                # --- the sequential scan ---
                for j in range(Tc):
                    h1 = sb[:, :, j]          # [65, 4]  ([h; 1])
                    hprev = sb[0:H, :, j]     # [64, 4]

                    psg = pg.tile([H, 2 * B_LOC], dt, tag="g")
                    psn = pn.tile([H, B_LOC], dt, tag="n")
                    # x-projection first: independent of h', so the PE runs it
                    # ahead; the h-matmuls are the only ones on the chain
                    nc.tensor.matmul(psg[:, :], wxrz[:, :], xz[:, :, j], start=True, stop=False, skip_group_check=True)
                    if split and j > 0:
                        # h_{j-1} = u + t3 (both produced in step j-1):
                        # accumulate W@u (available early) and W@t3 separately
                        ut, t3t = uts[(j - 1) % 2], t3s[(j - 1) % 2]
                        nc.tensor.matmul(psg[:, 0:B_LOC], Wr[:, :], ut[:, :], start=False, stop=False, skip_group_check=True)
                        nc.tensor.matmul(psg[:, B_LOC : 2 * B_LOC], Wz[:, :], ut[:, :], start=False, stop=False, skip_group_check=True)
                        nc.tensor.matmul(psn[:, :], Wn[:, :], ut[:, :], start=True, stop=False)
                        nc.tensor.matmul(psg[:, 0:B_LOC], Wr[0:H, :], t3t[:, :], start=False, stop=True, skip_group_check=True)
                        nc.tensor.matmul(psg[:, B_LOC : 2 * B_LOC], Wz[0:H, :], t3t[:, :], start=False, stop=True, skip_group_check=True)
                        nc.tensor.matmul(psn[:, :], Wn[0:H, :], t3t[:, :], start=False, stop=True)
                    else:
                        nc.tensor.matmul(psg[:, 0:B_LOC], Wr[:, :], h1, start=False, stop=True, skip_group_check=True)
                        nc.tensor.matmul(psg[:, B_LOC : 2 * B_LOC], Wz[:, :], h1, start=False, stop=True, skip_group_check=True)
                        nc.tensor.matmul(psn[:, :], Wn[:, :], h1, start=True, stop=True)

                    rz = wpool.tile([H, 2 * B_LOC], dt, tag="rz")
                    nc.scalar.activation(rz[:, :], psg[:, :], Act.Sigmoid)
                    # zp = 1 - z = sigmoid(-pre_z): on ACT, runs in the gap
                    # between sigmoid and tanh while DVE computes m/a
                    zp = wpool.tile([H, B_LOC], dt, tag="zp")
                    nc.scalar.activation(zp[:, :], psg[:, B_LOC : 2 * B_LOC], Act.Sigmoid, scale=-1.0)

                    if _STRIP == 2:
                        n_ = wpool.tile([H, B_LOC], dt, tag="nn")
                        nc.scalar.activation(n_[:, :], psn[:, :], Act.Tanh)
                    else:
                        m_ = wpool.tile([H, B_LOC], dt, tag="m")
                        nc.vector.tensor_tensor(m_[:, :], rz[:, 0:B_LOC], psn[:, :], Alu.mult)
                        a_ = wpool.tile([H, B_LOC], dt, tag="a")
                        nc.vector.tensor_tensor(a_[:, :], m_[:, :], xpn[:, :, j], Alu.add)
                        n_ = wpool.tile([H, B_LOC], dt, tag="nn")
                        nc.scalar.activation(n_[:, :], a_[:, :], Act.Tanh)

                    if _STRIP in (1, 2):
                        nc.vector.tensor_copy(sb[0:H, :, j + 1], n_[:, :])
                    elif split:
                        u_, t3 = uts[j % 2], t3s[j % 2]
                        nc.vector.tensor_tensor(u_[0:H, :], rz[:, B_LOC : 2 * B_LOC], hprev, Alu.mult)
                        nc.vector.tensor_tensor(t3[:, :], n_[:, :], zp[:, :], Alu.mult)
                        # h_j materialized off the critical chain (states + hprev)
                        nc.vector.tensor_tensor(sb[0:H, :, j + 1], t3[:, :], u_[0:H, :], Alu.add)
                    else:
                        u_ = wpool.tile([H, B_LOC], dt, tag="u")
                        nc.vector.tensor_tensor(u_[:, :], rz[:, B_LOC : 2 * B_LOC], hprev, Alu.mult)
                        t3 = wpool.tile([H, B_LOC], dt, tag="t3")
                        nc.vector.tensor_tensor(t3[:, :], n_[:, :], zp[:, :], Alu.mult)
                        nc.vector.tensor_tensor(sb[0:H, :, j + 1], t3[:, :], u_[:, :], Alu.add)

                # --- linear head: out = w_lin . h + b_lin + x ---
                for b in range(B_LOC):
                    ps = pm.tile([1, Tc], dt, tag="misc")
                    nc.tensor.matmul(ps[:, :], wlin[:, :], sb[0:H, b, 1 : Tc + 1], start=True, stop=False)
                    nc.tensor.matmul(ps[:, :], wxo[:, :], xt[:, b, :], start=False, stop=True)
                    osb = stpool.tile([1, Tc], dt, tag="osb")
                    nc.vector.tensor_copy(osb[:, :], ps[:, :])
                    nc.sync.dma_start(out_d[b : b + 1, bass.ds(t0, Tc)], osb[:, :])

                # --- states out: transpose [64, 128] -> [128, 64] and store ---
                for b in range(B_LOC):
                    for k in range(Tc // 128):
                        ps = pm.tile([128, H], dt, tag="misc")
                        nc.tensor.transpose(ps[:, :], sb[0:H, b, 1 + 128 * k : 1 + 128 * (k + 1)], ident[:, :])
                        ssb = stpool.tile([128, H], dt, tag="ssb")
                        if (b * (Tc // 128) + k) % 2 == 0:
                            nc.vector.tensor_copy(ssb[:, :], ps[:, :])
                        else:
                            nc.scalar.copy(ssb[:, :], ps[:, :])
                        nc.sync.dma_start(st_d[b, bass.ds(t0 + 128 * k, 128), :], ssb[:, :])

                # --- carry h across chunks ---
                nc.vector.tensor_copy(sb[0:H, :, 0], sb[0:H, :, Tc])

            if unroll:
                assert epochs == 1
                for t0 in range(0, T, Tc):
                    chunk_body(t0)
            else:
                ep_ctx = tc.For_i(0, epochs, 1) if epochs > 1 else None
                if ep_ctx is not None:
                    ep_ctx.__enter__()
                with tc.For_i(0, T, Tc) as t0:
                    chunk_body(t0)
                if ep_ctx is not None:
                    ep_ctx.__exit__(None, None, None)
            if internal_out:
                okt = cpool.tile([1, 1], dt, tag="okt")
                nc.vector.memset(okt[:, :], 1.0)
                nc.sync.dma_start(ok_d[:, :], okt[:, :])

    nc.compile()
    return nc


def _get_nc(T, Tc):
    key = (T, Tc)
    if key not in _cache:
        _cache[key] = _build(T, Tc, **BEST_KW)
    return _cache[key]


def _prep_weights(w_ih, w_hh, b_ih, b_hh, w_lin, b_lin):
    w_ih = np.asarray(w_ih, np.float32)
    w_hh = np.asarray(w_hh, np.float32)
    b_ih = np.asarray(b_ih, np.float32)
    b_hh = np.asarray(b_hh, np.float32)
    w_lin = np.asarray(w_lin, np.float32)
    b_lin = np.asarray(b_lin, np.float32)

    def lhsT(rows, bias):
        # [H+1, H]: rows 0..H-1 = W^T (W = w_hh[rows]), row H = bias
        m = np.empty((H + 1, H), np.float32)
        m[0:H, :] = w_hh[rows, :].T
        m[H, :] = bias
        return m

    r, z, n = slice(0, H), slice(H, 2 * H), slice(2 * H, 3 * H)
    d = {
        "lhsT_r": lhsT(r, b_ih[r] + b_hh[r]),
        "lhsT_z": lhsT(z, b_ih[z] + b_hh[z]),
        "lhsT_n": lhsT(n, b_hh[n]),
        "w_xrz": np.stack([w_ih[r, 0], w_ih[z, 0]]).astype(np.float32),
        "lhsT_xn": np.stack([w_ih[n, 0], b_ih[n]]).astype(np.float32),
        "w_lin_col": w_lin[0][:, None].copy(),
        "lhsT_xout": np.array([[1.0], [b_lin[0]]], np.float32),
        "ident": np.eye(H, dtype=np.float32),
    }
    return {k: np.ascontiguousarray(v) for k, v in d.items()}


def _run(x, weights, T, Tc):
    from concourse.bass_utils import run_bass_kernel_spmd

    nc = _get_nc(T, Tc)
    in_maps = []
    for c in range(N_CORES):
        m = {"x": np.ascontiguousarray(x[c * B_LOC : (c + 1) * B_LOC])}
        m.update(weights)
        in_maps.append(m)
    res = run_bass_kernel_spmd(nc, in_maps, list(range(N_CORES)))
    out = np.concatenate([res.results[c]["out"] for c in range(N_CORES)], axis=0)
    states = np.concatenate([res.results[c]["states"] for c in range(N_CORES)], axis=0)
    return out, states


def kernel(x, w_ih, w_hh, b_ih, b_hh, w_lin, b_lin, _Tc=512):
    x = np.asarray(x, np.float32)
    T = x.shape[1]
    weights = _prep_weights(w_ih, w_hh, b_ih, b_hh, w_lin, b_lin)
    return _run(x, weights, T, _Tc)
